# revision 32
# baseline (speedup 1.0000x reference)
"""Trainium2 Bass kernel for nn_Encoder_z0_ODE_RNN_causal_single_att.

Strategy:
- Data-parallel over n_traj: 512 trajectories -> 8 cores x 64.
- Everything on-chip lives in TRANSPOSED-PACKED layout: a [F, m] tensor
  (features x batch, F = 256 = 2 chunks of 128, m = 64) is stored as an SBUF
  tile [128, 2*64] where col block c holds feature rows [c*128,(c+1)*128).
- All matmuls are weights-stationary: out.T = W_chunk.T @ x.T, so no
  transposes are ever needed on-chip.
- Matmuls are fp32: the 200-step recurrence amplifies per-step rounding by
  ~300x, so bf16/tf32 operands diverge (measured); only the attention-e path
  (a feed-forward output) tolerates bf16.
- sigmoid(x) = 0.5 + 0.5*tanh(0.5*x): single ACT table set (exp+tanh).
- First-layer gate biases ride free on a constant-ones partition row appended
  to the xi chunk of the stationary weights.
- Attention accumulators: device emits per-step e_t rows and db_t = bcast(e_t)
  * y_t tiles; host does the fp32 cumulative sums and the b/a division during
  unsharding.
- Outputs staged in SBUF and DMA'd in 8-step blocks to partition-contiguous
  DRAM ([128, T*128]) to minimize descriptor count.
"""

import os
import sys

sys.path.insert(0, "/opt/trn_rl_repo")

import numpy as np
import ml_dtypes

import concourse.bass as bass
import concourse.mybir as mybir
import concourse.tile as tile
from concourse import bacc
from concourse.bass_utils import run_bass_kernel_spmd

N_TRAJ, N_TP, D_IN, L, U, H_ODE = 512, 200, 64, 256, 256, 256
NCORES = 8
M = N_TRAJ // NCORES  # 64 trajectories per core
T = int(os.environ.get("KERNEL_T", str(N_TP)))
TB = 8  # output DMA staging block (steps)
F32 = mybir.dt.float32
BF16 = mybir.dt.bfloat16
NPBF = ml_dtypes.bfloat16
AF = mybir.ActivationFunctionType
ALU = mybir.AluOpType

# Weight pack layout (columns of the packed [128, WCOLS] weights input).
# A weight W[K, Mout] is stored K-chunk-major:
# cols [k*Mout + c*128 : k*Mout + (c+1)*128] = W[k*128:(k+1)*128, c*128:(c+1)*128]
_off = 0


def _span(ncols):
    global _off
    s = _off
    _off += ncols
    return s


OFF_ODE1 = _span(512)   # w_ode1 [256,256]: 2 K-chunks
OFF_ODE2 = _span(512)   # w_ode2
OFF_U1 = _span(768)     # wu1 [320,256]: 3 K-chunks (last: 64 xi rows + bias row)
OFF_R1 = _span(768)
OFF_N1 = _span(768)     # first 256 rows pre-scaled by 0.5 (absorbs rg=0.5*(1+th))
OFF_U2 = _span(512)
OFF_R2 = _span(512)
OFF_N2 = _span(512)
WCOLS = _off


def _pack_l2(w):
    return np.concatenate([w[0:128, :], w[128:256, :]], axis=1)


def _pack_l1(w, b, scale_main=1.0):
    k2 = np.zeros((128, 256), np.float32)
    k2[0:64, :] = w[256:320, :]
    k2[64, :] = b
    return np.concatenate(
        [scale_main * w[0:128, :], scale_main * w[128:256, :], k2], axis=1
    )


def pack_weights(inp):
    w = np.zeros((128, WCOLS), np.float32)
    w[:, OFF_ODE1:OFF_ODE1 + 512] = _pack_l2(inp["w_ode1"])
    w[:, OFF_ODE2:OFF_ODE2 + 512] = _pack_l2(inp["w_ode2"])
    w[:, OFF_U1:OFF_U1 + 768] = _pack_l1(inp["wu1"], inp["bu1"])
    w[:, OFF_R1:OFF_R1 + 768] = _pack_l1(inp["wr1"], inp["br1"])
    w[:, OFF_N1:OFF_N1 + 768] = _pack_l1(inp["wn1"], inp["bn1"], scale_main=0.5)
    w[:, OFF_U2:OFF_U2 + 512] = _pack_l2(inp["wu2"])
    w[:, OFF_R2:OFF_R2 + 512] = _pack_l2(inp["wr2"])
    w[:, OFF_N2:OFF_N2 + 512] = _pack_l2(inp["wn2"])
    return w


def _bias128(b):
    out = np.zeros((128, 128), np.float32)
    out[:, 0:64] = b[0:128, None]
    out[:, 64:128] = b[128:256, None]
    return out


def build_nc(bias_flags):
    nc = bacc.Bacc("TRN2", target_bir_lowering=False, debug=False,
                   num_devices=NCORES, enable_asserts=False)

    d_w = nc.dram_tensor("wpack", [128, WCOLS], F32, kind="ExternalInput")
    d_watt = nc.dram_tensor("watt", [128, 2], F32, kind="ExternalInput")
    d_data = nc.dram_tensor("data_t", [64, T * M], F32, kind="ExternalInput")
    d_masks = nc.dram_tensor("masks", [1, T * M], F32, kind="ExternalInput")
    d_dts = nc.dram_tensor("dts", [1, T], F32, kind="ExternalInput")
    d_bias = {}
    for k in ("ode1", "ode2", "u2", "r2", "n2"):
        if bias_flags.get(k):
            d_bias[k] = nc.dram_tensor(f"bias_{k}", [128, 128], F32,
                                       kind="ExternalInput")

    d_lat = nc.dram_tensor("lat", [128, T * 128], F32, kind="ExternalOutput")
    d_db = nc.dram_tensor("db", [128, T * 128], F32, kind="ExternalOutput")
    d_e = nc.dram_tensor("ev", [1, T * M], BF16, kind="ExternalOutput")

    import contextlib

    with tile.TileContext(nc) as tc, contextlib.ExitStack() as ctx:
        consts = ctx.enter_context(tc.tile_pool(name="consts", bufs=1))
        stg = ctx.enter_context(tc.tile_pool(name="stg", bufs=2))
        sbf = ctx.enter_context(tc.tile_pool(name="sbf", bufs=2))
        work = ctx.enter_context(tc.tile_pool(name="work", bufs=3))
        psum = ctx.enter_context(tc.tile_pool(name="psum", bufs=1, space="PSUM"))

        # ---- one-time loads ----
        w_sb = consts.tile([128, WCOLS], F32)
        nc.sync.dma_start(out=w_sb, in_=d_w[:, :])
        watt_sb = consts.tile([128, 2], F32)
        nc.sync.dma_start(out=watt_sb, in_=d_watt[:, :])
        data_sb = consts.tile([65, T * M], F32)
        nc.sync.dma_start(out=data_sb[0:64, :], in_=d_data[:, :])
        nc.vector.memset(data_sb[64:65, :], 1.0)  # ones row (bias folding)
        masks_sb = consts.tile([128, T * M], F32)
        nc.sync.dma_start(
            out=masks_sb,
            in_=bass.AP(tensor=d_masks.ap().tensor, offset=d_masks.ap().offset,
                        ap=[[0, 128]] + [d_masks.ap().ap[1]]),
        )
        dts_sb = consts.tile([128, T], F32)
        nc.sync.dma_start(
            out=dts_sb,
            in_=bass.AP(tensor=d_dts.ap().tensor, offset=d_dts.ap().offset,
                        ap=[[0, 128]] + [d_dts.ap().ap[1]]),
        )
        ones_sb = consts.tile([1, 128], BF16)
        nc.vector.memset(ones_sb, 1.0)
        e_stage = consts.tile([1, T * M], BF16)
        bias_sb = {}
        for k, dt_ in d_bias.items():
            bias_sb[k] = consts.tile([128, 128], F32, tag=f"bias_{k}")
            nc.sync.dma_start(out=bias_sb[k], in_=dt_[:, :])

        # ---- state ----
        ytd_f = sbf.tile([128, 128], F32, tag="y0")
        nc.vector.memset(ytd_f, 0.0)

        stage_lat = stage_db = None
        for t in range(T):
            s = t % TB
            if s == 0:
                stage_lat = stg.tile([128, TB * 128], F32, tag="slat")
                stage_db = stg.tile([128, TB * 128], F32, tag="sdb")
            dtc = dts_sb[:, t:t + 1]
            xi = data_sb[0:65, t * M:(t + 1) * M]
            yc = [ytd_f[:, 0:64], ytd_f[:, 64:128]]

            def wchunk(woff, k, c, rows=128):
                return w_sb[0:rows, woff + k * 256 + c * 128:
                            woff + k * 256 + (c + 1) * 128]

            # ODE layer 1 (per-chunk PSUM banks) + attention + g1-xi prologue
            h1p = [psum.tile([128, 64], F32, tag="pA", name=f"h1p0_{t}"),
                   psum.tile([128, 64], F32, tag="pB", name=f"h1p1_{t}")]
            for c in range(2):
                for k in range(2):
                    nc.tensor.matmul(h1p[c], wchunk(OFF_ODE1, k, c),
                                     yc[k], start=(k == 0), stop=(k == 1))
            attp = psum.tile([1, 64], F32, tag="pE", bufs=2, name=f"attp_{t}")
            for k in range(2):
                nc.tensor.matmul(attp, watt_sb[:, k:k + 1], yc[k],
                                 start=(k == 0), stop=(k == 1))
            g1u = psum.tile([128, 128], F32, tag="pC", bufs=2, name=f"g1u_{t}")
            g1r = [psum.tile([128, 64], F32, tag="pD0", name=f"g1r0_{t}"),
                   psum.tile([128, 64], F32, tag="pD1", name=f"g1r1_{t}")]

            # h1 tanh per chunk; attention exp
            h1t = []
            for c in range(2):
                src = h1p[c]
                if "ode1" in bias_sb:
                    hb = work.tile([128, 64], F32, tag=f"h1b{c}")
                    nc.vector.tensor_add(hb, h1p[c],
                                         bias_sb["ode1"][:, c * 64:(c + 1) * 64])
                    src = hb
                ht = work.tile([128, 64], F32, tag=f"h1t{c}", name=f"h1t{c}_{t}")
                nc.scalar.activation(ht, src, AF.Tanh)
                h1t.append(ht)
            e_row = e_stage[0:1, t * M:(t + 1) * M]

            # ODE layer 2
            dyp = [psum.tile([128, 64], F32, tag="pA", name=f"dyp0_{t}"),
                   psum.tile([128, 64], F32, tag="pB", name=f"dyp1_{t}")]
            for k in range(2):
                for c in range(2):
                    nc.tensor.matmul(dyp[c], wchunk(OFF_ODE2, k, c),
                                     h1t[k], start=(k == 0), stop=(k == 1))

            # yi = y + dt*dy, per chunk
            yi = []
            for c in range(2):
                yv = work.tile([128, 64], F32, tag=f"yi{c}", name=f"yi{c}_{t}")
                nc.vector.scalar_tensor_tensor(yv, dyp[c], dtc, yc[c],
                                               ALU.mult, ALU.add)
                if "ode2" in bias_sb:
                    nc.vector.scalar_tensor_tensor(
                        yv, bias_sb["ode2"][:, c * 64:(c + 1) * 64], dtc,
                        yv, ALU.mult, ALU.add)
                yi.append(yv)

            # g1 yi contributions
            for c in range(2):
                nc.tensor.matmul(g1r[c], wchunk(OFF_R1, 2, c, rows=65),
                                 xi, start=True, stop=False)
                for k in range(2):
                    nc.tensor.matmul(g1r[c], wchunk(OFF_R1, k, c), yi[k],
                                     start=False, stop=(k == 1))
            for c in range(2):
                nc.tensor.matmul(g1u[:, c * 64:(c + 1) * 64],
                                 wchunk(OFF_U1, 2, c, rows=65),
                                 xi, start=True, stop=False)
                for k in range(2):
                    nc.tensor.matmul(g1u[:, c * 64:(c + 1) * 64],
                                     wchunk(OFF_U1, k, c), yi[k],
                                     start=False, stop=(k == 1))
            g1tu = work.tile([128, 128], F32, tag="g1tu", name=f"g1tu_{t}")
            g1tr = [work.tile([128, 64], F32, tag="g1tr0", name=f"g1tr0_{t}"),
                    work.tile([128, 64], F32, tag="g1tr1", name=f"g1tr1_{t}")]
            nc.scalar.activation(g1tr[0], g1r[0], AF.Tanh)
            nc.scalar.activation(g1tr[1], g1r[1], AF.Tanh)
            nc.scalar.activation(g1tu, g1u, AF.Tanh)

            # gate layer 2 + sigmoid-as-tanh
            g2u = psum.tile([128, 128], F32, tag="pC", bufs=2, name=f"g2u_{t}")
            g2r = [psum.tile([128, 64], F32, tag="pD0", name=f"g2r0_{t}"),
                   psum.tile([128, 64], F32, tag="pD1", name=f"g2r1_{t}")]
            n1p = [psum.tile([128, 64], F32, tag="pA", name=f"n1p0_{t}"),
                   psum.tile([128, 64], F32, tag="pB", name=f"n1p1_{t}")]
            for c in range(2):
                for k in range(2):
                    nc.tensor.matmul(g2r[c], wchunk(OFF_R2, k, c),
                                     g1tr[k],
                                     start=(k == 0), stop=(k == 1))
            for c in range(2):
                nc.tensor.matmul(n1p[c], wchunk(OFF_N1, 2, c, rows=65),
                                 xi, start=True, stop=False)
            for c in range(2):
                for k in range(2):
                    nc.tensor.matmul(g2u[:, c * 64:(c + 1) * 64],
                                     wchunk(OFF_U2, k, c),
                                     g1tu[:, k * 64:(k + 1) * 64],
                                     start=(k == 0), stop=(k == 1))
            thu = work.tile([128, 128], F32, tag="thu", name=f"thu_{t}")
            thr = [work.tile([128, 64], F32, tag="thr0", name=f"thr0_{t}"),
                   work.tile([128, 64], F32, tag="thr1", name=f"thr1_{t}")]
            if "u2" in bias_sb:
                for c in range(2):
                    grb = work.tile([128, 64], F32, tag=f"grb{c}")
                    nc.vector.tensor_add(grb, g2r[c],
                                         bias_sb["r2"][:, c * 64:(c + 1) * 64])
                    nc.scalar.activation(thr[c], grb, AF.Tanh, scale=0.5)
                gub = work.tile([128, 128], F32, tag="gub")
                nc.vector.tensor_add(gub, g2u, bias_sb["u2"])
                nc.scalar.activation(thu, gub, AF.Tanh, scale=0.5)
            else:
                nc.scalar.activation(thr[0], g2r[0], AF.Tanh, scale=0.5)
                nc.scalar.activation(thr[1], g2r[1], AF.Tanh, scale=0.5)
                nc.scalar.activation(thu, g2u, AF.Tanh, scale=0.5)

            # n-gate: yirg then layer-1 yirg contributions
            yirg = []
            for c in range(2):
                yr = work.tile([128, 64], F32, tag=f"yirg{c}", name=f"yirg{c}_{t}")
                nc.vector.scalar_tensor_tensor(yr, thr[c],
                                               1.0, yi[c], ALU.add, ALU.mult)
                yirg.append(yr)
            for k in range(2):
                for c in range(2):
                    nc.tensor.matmul(n1p[c], wchunk(OFF_N1, k, c), yirg[k],
                                     start=False, stop=(k == 1))
            n1t = []
            for c in range(2):
                nt = work.tile([128, 64], F32, tag=f"n1t{c}", name=f"n1t{c}_{t}")
                nc.scalar.activation(nt, n1p[c], AF.Tanh)
                n1t.append(nt)
            nc.scalar.activation(e_row, attp, AF.Exp)
            nsp = [psum.tile([128, 64], F32, tag="pD0", name=f"nsp0_{t}"),
                   psum.tile([128, 64], F32, tag="pD1", name=f"nsp1_{t}")]
            for k in range(2):
                for c in range(2):
                    nc.tensor.matmul(nsp[c], wchunk(OFF_N2, k, c), n1t[k],
                                     start=(k == 0), stop=(k == 1))
            ebp = psum.tile([128, 128], F32, tag="pE", bufs=2, name=f"ebp_{t}")
            e_dup = bass.AP(tensor=e_row.tensor, offset=e_row.offset,
                            ap=[e_row.ap[0], [0, 2], [1, 64]])
            nc.tensor.matmul(ebp, ones_sb, e_dup, start=True, stop=True)

            # blend: ynew = yi + Bm*(ns - yi), Bm = mask*(.5-.5*th_u)
            Bm = work.tile([128, 128], F32, tag="Bm", name=f"Bm_{t}")
            nc.vector.tensor_scalar(Bm, thu, -0.5, 0.5, ALU.mult, ALU.add)
            for c in range(2):
                nc.vector.tensor_mul(Bm[:, c * 64:(c + 1) * 64],
                                     Bm[:, c * 64:(c + 1) * 64],
                                     masks_sb[:, t * M:(t + 1) * M])
            Am = work.tile([128, 128], F32, tag="Am", name=f"Am_{t}")
            nc.vector.tensor_scalar(Am, Bm, -1.0, 1.0, ALU.mult, ALU.add)
            yiA = []
            for c in range(2):
                ya = work.tile([128, 64], F32, tag=f"yiA{c}", name=f"yiA{c}_{t}")
                nc.vector.tensor_mul(ya, yi[c], Am[:, c * 64:(c + 1) * 64])
                yiA.append(ya)
            for c in range(2):
                nsin = nsp[c]
                if "n2" in bias_sb:
                    nsb = work.tile([128, 64], F32, tag=f"nsb{c}")
                    nc.vector.tensor_add(nsb, nsp[c],
                                         bias_sb["n2"][:, c * 64:(c + 1) * 64])
                    nsin = nsb
                q1 = work.tile([128, 64], F32, tag=f"q1{c}", name=f"q1{c}_{t}")
                nc.vector.tensor_mul(q1, nsin, Bm[:, c * 64:(c + 1) * 64])
                nc.vector.tensor_add(stage_lat[:, s * 128 + c * 64:
                                               s * 128 + (c + 1) * 64],
                                     q1, yiA[c])

            nc.vector.tensor_mul(stage_db[:, s * 128:(s + 1) * 128],
                                 ebp[:, 0:128], ytd_f[:, 0:128])

            if s == TB - 1 or t == T - 1:
                t0 = (t // TB) * TB
                w_ = (t - t0 + 1) * 128
                nc.sync.dma_start(out=d_lat[:, t0 * 128: t0 * 128 + w_],
                                  in_=stage_lat[:, 0:w_])
                nc.gpsimd.dma_start(out=d_db[:, t0 * 128: t0 * 128 + w_],
                                    in_=stage_db[:, 0:w_])

            ytd_f = stage_lat[:, s * 128:(s + 1) * 128]

        nc.sync.dma_start(out=d_e[:, :], in_=e_stage)

    nc.finalize()
    return nc


_CACHE = {}
_LAST_RESULT = None


def _get_nc(bias_flags):
    key = tuple(sorted(bias_flags.items()))
    if key not in _CACHE:
        _CACHE[key] = build_nc(bias_flags)
    return _CACHE[key]


def prepare(inp):
    g2bias = bool(np.any(inp["bu2"])) or bool(np.any(inp["br2"]))
    bias_flags = {
        "ode1": bool(np.any(inp["b_ode1"])),
        "ode2": bool(np.any(inp["b_ode2"])),
        "u2": g2bias,
        "r2": g2bias,
        "n2": bool(np.any(inp["bn2"])),
    }
    wpack = pack_weights(inp)
    watt = np.zeros((128, 2), np.float32)
    watt[:, 0] = inp["w_att"][0:128, 0]
    watt[:, 1] = inp["w_att"][128:256, 0]
    ts = inp["time_steps"]
    dts = np.concatenate([np.full((1,), 0.01, np.float32),
                          np.diff(ts)]).astype(np.float32)[:T]

    in_maps = []
    for ci in range(NCORES):
        dc = inp["data"][ci * M:(ci + 1) * M, :T, :]   # [64, T, 64]
        data_t = np.ascontiguousarray(dc.transpose(2, 1, 0)).reshape(64, T * M)
        masks = (dc[:, :, D_IN - 1] > 0).astype(np.float32)  # [64, T]
        masks_r = np.ascontiguousarray(masks.T).reshape(1, T * M)
        im = {
            "wpack": wpack,
            "watt": watt,
            "data_t": data_t,
            "masks": masks_r,
            "dts": dts.reshape(1, T),
        }
        for k, arr in (("ode1", inp["b_ode1"]), ("ode2", inp["b_ode2"]),
                       ("u2", inp["bu2"]), ("r2", inp["br2"]),
                       ("n2", inp["bn2"])):
            if bias_flags[k]:
                im[f"bias_{k}"] = _bias128(arr)
        in_maps.append(im)
    return bias_flags, in_maps, dts


def postprocess(res, dts):
    latent = np.empty((N_TRAJ, T, L), np.float32)
    ctx = np.empty((N_TRAJ, T, L), np.float32)
    for ci in range(NCORES):
        lat_c = res.results[ci]["lat"].reshape(128, T, 2, M)
        db_c = res.results[ci]["db"].reshape(128, T, 2, M)
        e_c = res.results[ci]["ev"].astype(np.float32).reshape(T, M)
        # latent[j, t, c*128+p] = lat_c[p, t, c, j]
        latent[ci * M:(ci + 1) * M] = (
            lat_c.transpose(3, 1, 2, 0).reshape(M, T, L))
        a = np.cumsum(e_c * dts[:, None], axis=0)               # [T, M]
        b = np.cumsum(db_c * dts[None, :, None, None], axis=1)  # [128,T,2,M]
        ctxc = b / a[None, :, None, :]
        ctx[ci * M:(ci + 1) * M] = ctxc.transpose(3, 1, 2, 0).reshape(M, T, L)
    mean_z0 = np.ascontiguousarray(latent[:, -1, :])
    return mean_z0, latent, ctx


def kernel(data, time_steps, w_ode1, b_ode1, w_ode2, b_ode2, w_att,
           wu1, bu1, wu2, bu2, wr1, br1, wr2, br2, wn1, bn1, wn2, bn2):
    inp = dict(data=np.asarray(data, np.float32),
               time_steps=np.asarray(time_steps, np.float32))
    for k, v in dict(w_ode1=w_ode1, b_ode1=b_ode1, w_ode2=w_ode2, b_ode2=b_ode2,
                     w_att=w_att, wu1=wu1, bu1=bu1, wu2=wu2, bu2=bu2, wr1=wr1,
                     br1=br1, wr2=wr2, br2=br2, wn1=wn1, bn1=bn1, wn2=wn2,
                     bn2=bn2).items():
        inp[k] = np.asarray(v, np.float32)
    bias_flags, in_maps, dts = prepare(inp)
    nc = _get_nc(bias_flags)
    res = run_bass_kernel_spmd(nc, in_maps, core_ids=list(range(NCORES)))
    global _LAST_RESULT
    _LAST_RESULT = res
    return postprocess(res, dts)
